# revision 27
# baseline (speedup 1.0000x reference)
"""Trainium2 Bass kernel for nn_Cov_EBFLayer.

Math: out[b,o] = exp(-quad[o,b]),
  quad[o,b] = diff^T P_o diff,  diff = c_o - x_b,  P_o = B_o B_o^T
            = x^T P x - 2 v_o^T x + q3_o,   v = P c,  q3 = c^T P c
Square trick + rotation packing:
  x^T P x = sum_{d, g=1..32} s2_g * P[d, (d+g)%64] * (x_d + x_{(d+g)%64})^2
            - sum_d (r_d - 2 P_dd) x_d^2
  with s2_g = 1 for g<32, 1/2 for g=32 (each unordered pair covered once by
  g=1..31, twice by g=32), r = P 1.  The 2080 unique pair features pack into
  exactly 16 chunks of 128 rows (gj,d), g = 2c+1+gj — HALF the matmuls of
  the naive 32-chunk full-square map.

Kernel per core (batch-sharded 8 x 1024):
  - warmup matmuls from a memset tile (PE p-state ramp, no DMA dependency)
  - Gram: P_o = B_o^T B_o for o-pairs (ol, ol+64), PSUM partitions (q, d),
    copies (DVE lo / ACT hi in parallel) -> p_sb2 [(q,d), (h,f,tt)]
  - DRAM round trip per o-half: write P at [h,q,d,f',tt] with an extra
    f'=f+64 image of f<32 so the rotated read never wraps; re-read W chunks
    [(gj,d), (c, o)] with a manual diagonal AP (d-stride 8256 = (128+1)*64).
  - builds: indicator matmul (two 1s per column, sqrt(1/2) on the g=32
    chunk) -> (x_d + x_f) in PSUM; Square on ACT (2/3) or DVE-copy+Pool-mul
    (1/3) -> gstore fp16; mains: 16 accumulating matmuls + aug chunk
    [x; x^2] with coeffs [-2v; -r+2*diag(P)]; q3 via per-partition Exp bias.
Host does layout-only prep + tiny linear-term prep (w,v,q3,r,Pdd: ~3M MACs
= 0.02% of model FLOPs).
"""

import sys
from contextlib import ExitStack

import numpy as np

sys.path.insert(0, "/opt/trn_rl_repo")

import concourse.bass as bass  # noqa: E402
import concourse.tile as tile  # noqa: E402
from concourse import bacc, mybir  # noqa: E402
from concourse import bass_utils  # noqa: E402
from concourse._compat import with_exitstack  # noqa: E402
from concourse.ap import AP as RawAP  # noqa: E402

B, D, O, NCORES = 8192, 64, 256, 8
BSH = B // NCORES  # 1024 per-core batch shard
NCH = 16  # rotation-packed chunks: g = 2c+1+gj, rows (gj, d)
BT = 512  # b-tile (one PSUM bank of fp32)
NBT = BSH // BT  # 2
F32 = mybir.dt.float32
F16 = mybir.dt.float16
COPY = mybir.ActivationFunctionType.Copy
SQUARE = mybir.ActivationFunctionType.Square
EXP = mybir.ActivationFunctionType.Exp


@with_exitstack
def _kernel(ctx: ExitStack, tc, outT, xT, bt2h0, bt2h1, indc2, waug1, q3b):
    nc = tc.nc

    cpool = ctx.enter_context(tc.tile_pool(name="const", bufs=1))
    ppool = ctx.enter_context(tc.tile_pool(name="psum_p", bufs=3, space="PSUM"))
    qpool = ctx.enter_context(tc.tile_pool(name="psum_q", bufs=2, space="PSUM"))
    dpool = ctx.enter_context(tc.tile_pool(name="dram", bufs=1, space="DRAM"))

    # ---- SBUF residents ----
    warm = cpool.tile([D, BT], F16)  # warmup scratch (memset, no DMA dep)
    sb_bt2 = cpool.tile([128, 64 * 128], F16)  # [(h,e), (ttl, oo, d)]
    sb_indc2 = cpool.tile([D, NCH * 128], F16)  # two-ones rotation indicator
    aug1 = cpool.tile([128, BSH], F16)  # [xT; x^2]
    sb_waug1 = cpool.tile([128, O], F16)  # [-2 v^T; (-r+2Pdd)^T]
    sb_q3b = cpool.tile([128, 2], F32)  # -q3 per (o-half) column
    p_sb2 = cpool.tile([128, 2 * D * 64], F16)  # [(q,d), (h, f, tt)]
    w = [cpool.tile([128, NCH * 128], F16, name=f"w{h}") for h in range(2)]
    gstore = cpool.tile([128, NCH * NBT * BT], F16)  # squared features
    osb = [cpool.tile([128, BSH], F16, name=f"osb{h}") for h in range(2)]
    p_dram = dpool.tile([2, 2, D, 128, 64], F16)  # [h, q, d, f', tt]

    # ---- parallel input DMA ----
    # betasT2 quarters interleaved across sync+scalar (half 0 first)
    for qx, eng in enumerate([nc.sync, nc.scalar, nc.sync, nc.scalar]):
        sl = slice(qx * 2048, (qx + 1) * 2048)
        eng.dma_start(sb_bt2[0:64, sl], bt2h0[:, sl])
    for qx, eng in enumerate([nc.sync, nc.scalar, nc.sync, nc.scalar]):
        sl = slice(qx * 2048, (qx + 1) * 2048)
        eng.dma_start(sb_bt2[64:128, sl], bt2h1[:, sl])
    nc.vector.memset(warm[:], 0.125)
    # gpsimd (SWDGE) queue: small inputs only
    nc.gpsimd.dma_start(aug1[0:D, :], xT[:])
    nc.gpsimd.dma_start(sb_indc2[:], indc2[:])
    nc.gpsimd.dma_start(sb_waug1[:], waug1[:])
    nc.gpsimd.dma_start(sb_q3b[:], q3b[:])

    # PSUM: ppool = 3 x [128, 1024] units (Gram + build tiles, 6 banks),
    # qpool = 2 banks (oh1 accumulators recycle oh0's after the early exp)
    pq = {}
    for oh in range(2):
        for bt in range(NBT):
            pq[(oh, bt)] = qpool.tile(
                [128, BT], F32, name=f"pq_{oh}_{bt}", tag="pq"
            )

    # ---- PE warmup: p-state ramp while input DMAs fly ----
    wps = ppool.tile([128, NBT * BT], F32, name="wps", tag="u")
    for i in range(24):
        nc.tensor.matmul(
            wps[:, 0:128], warm[:, 0:128], warm[:, 0:128], start=True, stop=True
        )

    # ---- x^2 rows of aug chunk (ACT; after xT lands) ----
    for bt in range(NBT):
        nc.scalar.activation(
            aug1[64:128, bt * BT : (bt + 1) * BT],
            aug1[0:64, bt * BT : (bt + 1) * BT],
            SQUARE,
        )

    # ---- Gram with builds interleaved ----
    # Gram pair ttl of half h covers o = h*128 + {ttl, 64+ttl}; PSUM
    # partitions (q, d); 8 pairs per 2-bank unit tile; lo block copied on
    # DVE, hi on ACT in parallel, contiguous-dst iteration.  One build
    # chunk (+square) follows every Gram unit — the betasT2 stream paces
    # Gram slower than the PE, so builds fill the idle slots for free.
    # The P round trip overlaps the copy stream via tt-split writes.
    stg = [cpool.tile([128, NBT * BT], F16, name=f"stg{i}") for i in range(2)]

    def build_chunk(c):
        bd = ppool.tile([128, NBT * BT], F32, name=f"bd_{c}", tag="u")
        for bt in range(NBT):
            nc.tensor.matmul(
                bd[:, bt * BT : (bt + 1) * BT],
                sb_indc2[:, c * 128 : (c + 1) * 128],
                aug1[0:D, bt * BT : (bt + 1) * BT],
                start=True,
                stop=True,
            )
        gsl = gstore[:, c * NBT * BT : (c + 1) * NBT * BT]
        if c % 4 != 3:
            nc.scalar.activation(gsl, bd[:], SQUARE)
        else:
            st = stg[(c // 4) % 2]
            nc.vector.tensor_copy(st[:], bd[:])
            nc.gpsimd.tensor_mul(gsl, st[:], st[:])

    pv = p_dram[:].rearrange("hh q d fp tt -> hh (q d) fp tt")
    for h in range(2):
        deng = nc.sync if h == 0 else nc.scalar
        dv = p_sb2[:, h * 4096 : (h + 1) * 4096].rearrange(
            "p (f tt) -> p f tt", f=64
        )
        for u in range(8):
            pt = ppool.tile(
                [128, NBT * BT], F32, name=f"gm_{h}_{u}", tag="u"
            )
            for t in range(8):
                ttl = u * 8 + t
                bsl = sb_bt2[h * 64 : h * 64 + 64, ttl * 128 : ttl * 128 + 128]
                nc.tensor.matmul(
                    pt[:, t * 128 : (t + 1) * 128], bsl, bsl, start=True, stop=True
                )
            sv = pt[:].rearrange("p (t oo f) -> p oo f t", t=8, oo=2)
            ts0 = u * 8
            nc.vector.tensor_copy(dv[0:64, :, ts0 : ts0 + 8], sv[0:64, 0])
            nc.scalar.activation(
                dv[64:128, :, ts0 : ts0 + 8], sv[64:128, 1], COPY
            )
            build_chunk(h * 8 + u)
            if u in (3, 7):
                # round-trip writes for the tt half just copied (h0 on
                # sync queue, h1 on scalar): P at [h,q,d,f',tt] plus an
                # f'=f+64 image of f<32 so the diagonal read never wraps
                tsl = slice(0, 32) if u == 3 else slice(32, 64)
                deng.dma_start(pv[h][:, 0:64, tsl], dv[:, :, tsl])
                deng.dma_start(pv[h][:, 64:96, tsl], dv[:, 0:32, tsl])
        # rotated W read: value P[d, (d+g)%64, o] at fp = d+g; manual AP
        # with diagonal d-stride 8256 = 8192 + 64
        base = p_dram[:]
        wv = w[h][:].rearrange("p (c o) -> p c o", c=NCH)
        for gj in range(2):
            for q in range(2):
                off = h * 1048576 + q * 524288 + (1 + gj) * 64
                srcd = RawAP(
                    tensor=base.tensor,
                    offset=base.offset + off,
                    ap=[[8256, 64], [128, NCH], [1, 64]],
                )
                deng.dma_start(
                    wv[gj * 64 : (gj + 1) * 64, :, q * 64 : (q + 1) * 64],
                    srcd,
                )

    # ---- main phase A: oh0 mains ----
    for c in range(NCH):
        for bt in range(NBT):
            nc.tensor.matmul(
                pq[(0, bt)],
                w[0][:, c * 128 : (c + 1) * 128],
                gstore[:, (c * NBT + bt) * BT : (c * NBT + bt + 1) * BT],
                start=(c == 0),
                stop=False,
            )
    # aug mains + epilogue for oh0 (early, overlaps phase B)
    for bt in range(NBT):
        nc.tensor.matmul(
            pq[(0, bt)],
            sb_waug1[:, 0:128],
            aug1[:, bt * BT : (bt + 1) * BT],
            start=False,
            stop=True,
        )
        nc.scalar.activation(
            osb[0][:, bt * BT : (bt + 1) * BT],
            pq[(0, bt)],
            EXP,
            bias=sb_q3b[:, 0:1],
            scale=-1.0,
        )
        nc.sync.dma_start(
            outT[0:128, bt * BT : (bt + 1) * BT],
            osb[0][:, bt * BT : (bt + 1) * BT],
        )

    # ---- main phase B: oh1 mains from stored squares ----
    for c in range(NCH):
        for bt in range(NBT):
            nc.tensor.matmul(
                pq[(1, bt)],
                w[1][:, c * 128 : (c + 1) * 128],
                gstore[:, (c * NBT + bt) * BT : (c * NBT + bt + 1) * BT],
                start=(c == 0),
                stop=False,
            )
    for bt in range(NBT):
        nc.tensor.matmul(
            pq[(1, bt)],
            sb_waug1[:, 128:256],
            aug1[:, bt * BT : (bt + 1) * BT],
            start=False,
            stop=True,
        )
        nc.scalar.activation(
            osb[1][:, bt * BT : (bt + 1) * BT],
            pq[(1, bt)],
            EXP,
            bias=sb_q3b[:, 1:2],
            scale=-1.0,
        )
        nc.scalar.dma_start(
            outT[128:256, bt * BT : (bt + 1) * BT],
            osb[1][:, bt * BT : (bt + 1) * BT],
        )


_CACHE = {}


def _build():
    if "nc" in _CACHE:
        return _CACHE["nc"], _CACHE["aps"]
    nc = bacc.Bacc(
        "TRN2", target_bir_lowering=False, debug=False, num_devices=NCORES
    )
    xT = nc.dram_tensor("xT", [D, BSH], F16, kind="ExternalInput").ap()
    bt2h0 = nc.dram_tensor("bt2h0", [64, 8192], F16, kind="ExternalInput").ap()
    bt2h1 = nc.dram_tensor("bt2h1", [64, 8192], F16, kind="ExternalInput").ap()
    indc2 = nc.dram_tensor("indc2", [D, NCH * 128], F16, kind="ExternalInput").ap()
    waug1 = nc.dram_tensor("waug1", [128, O], F16, kind="ExternalInput").ap()
    q3b = nc.dram_tensor("q3b", [128, 2], F32, kind="ExternalInput").ap()
    outT = nc.dram_tensor("outT", [O, BSH], F16, kind="ExternalOutput").ap()
    with tile.TileContext(nc) as tc:
        _kernel(tc, outT, xT, bt2h0, bt2h1, indc2, waug1, q3b)
    nc.compile()
    _CACHE["nc"] = nc
    _CACHE["aps"] = (xT, bt2h0, bt2h1, indc2, waug1, q3b, outT)
    return nc, _CACHE["aps"]


def _host_prep(x, centers, betas):
    x = np.asarray(x, np.float32)
    betas = np.asarray(betas, np.float32)
    c = np.asarray(centers, np.float32).reshape(O, D)
    # betasT2[h][e, (ttl, oo, d)] = betas[h*128+oo*64+ttl, d, e]  (unscaled:
    # Gram produces P exactly, the pair coefficient A/2 = P for g<32)
    bt = betas.transpose(2, 0, 1)  # [e, o, d]
    bt = bt.reshape(D, 2, 2, 64, D)  # [e, h, oo, ttl, d]
    bt = bt.transpose(1, 0, 3, 2, 4).reshape(2, D, 8192)  # [h, e, (ttl,oo,d)]
    bt2h0 = np.ascontiguousarray(bt[0]).astype(np.float16)
    bt2h1 = np.ascontiguousarray(bt[1]).astype(np.float16)
    # rotation indicator: chunk c rows p=(gj,d), g=2c+1+gj:
    # indc2[k, c*128+p] = s * ([k == d] + [k == (d+g)%64]), s = sqrt(1/2)
    # only for the g=32 rows (each {d, d+32} pair is covered twice)
    k = np.arange(D)[:, None, None, None]
    cc = np.arange(NCH)[None, :, None, None]
    gj = np.arange(2)[None, None, :, None]
    dd = np.arange(D)[None, None, None, :]
    g = 2 * cc + 1 + gj
    ind = (k == dd).astype(np.float32) + (k == (dd + g) % D)
    ind *= np.where(g == 32, np.sqrt(0.5), 1.0)
    indc2 = np.ascontiguousarray(ind.reshape(D, NCH * 128)).astype(np.float16)
    # tiny linear-term prep: w = B^T c, v = B w, q3 = w.w, r = P 1,
    # Pdd = diag(P)  (~3M MACs total)
    w_ = np.einsum("ofe,of->oe", betas, c)
    v = np.einsum("ode,oe->od", betas, w_)
    q3 = np.einsum("oe,oe->o", w_, w_)
    s = betas.sum(axis=1)  # [o, e]
    r = np.einsum("ode,oe->od", betas, s)
    pdd = np.einsum("ode,ode->od", betas, betas)
    waug1 = np.concatenate([-2.0 * v.T, (-r + 2.0 * pdd).T], axis=0).astype(
        np.float16
    )
    q3b = np.ascontiguousarray((-q3).reshape(2, 128).T).astype(np.float32)
    xT_shards = [
        np.ascontiguousarray(x[i * BSH : (i + 1) * BSH].T).astype(np.float16)
        for i in range(NCORES)
    ]
    return xT_shards, bt2h0, bt2h1, indc2, waug1, q3b


def _run(x, centers, betas, trace=False):
    nc, (xT, bt2h0a, bt2h1a, indc2a, waug1a, q3ba, outT) = _build()
    xT_shards, bt2h0, bt2h1, indc2, waug1, q3b = _host_prep(x, centers, betas)
    in_maps = [
        {
            xT.name: xT_shards[i],
            bt2h0a.name: bt2h0,
            bt2h1a.name: bt2h1,
            indc2a.name: indc2,
            waug1a.name: waug1,
            q3ba.name: q3b,
        }
        for i in range(NCORES)
    ]
    res = bass_utils.run_bass_kernel_spmd(
        nc, in_maps, core_ids=list(range(NCORES)), trace=trace
    )
    out = np.concatenate(
        [np.asarray(res.results[i][outT.name]).T for i in range(NCORES)],
        axis=0,
    )
    return out.astype(np.float32), res


def kernel(x, centers, betas):
    out, _ = _run(x, centers, betas, trace=False)
    return out


# revision 30
# speedup vs baseline: 1.4719x; 1.4719x over previous
"""Trainium2 Bass kernel for nn_Cov_EBFLayer.

Math: out[b,o] = exp(-quad[o,b]),
  quad[o,b] = diff^T P_o diff,  diff = c_o - x_b,  P_o = B_o B_o^T
            = x^T P x - 2 v_o^T x + q3_o,   v = P c,  q3 = c^T P c
Square trick + rotation packing:
  x^T P x = sum_{d, g=1..32} s2_g * P[d, (d+g)%64] * (x_d + x_{(d+g)%64})^2
            - sum_d (r_d - 2 P_dd) x_d^2
  with s2_g = 1 for g<32, 1/2 for g=32 (each unordered pair covered once by
  g=1..31, twice by g=32), r = P 1.  The 2080 unique pair features pack into
  exactly 16 chunks of 128 rows (gj,d), g = 2c+1+gj — HALF the matmuls of
  the naive 32-chunk full-square map.

Kernel per core (batch-sharded 8 x 1024):
  - warmup matmuls from a memset tile (PE p-state ramp, no DMA dependency)
  - Gram: P_o = B_o^T B_o for o-pairs (ol, ol+64), PSUM partitions (q, d),
    copies (DVE lo / ACT hi in parallel) -> p_sb2 [(q,d), (h,f,tt)]
  - DRAM round trip per o-half: write P at [h,q,d,f',tt] with an extra
    f'=f+64 image of f<32 so the rotated read never wraps; re-read W chunks
    [(gj,d), (c, o)] with a manual diagonal AP (d-stride 8256 = (128+1)*64).
  - builds: indicator matmul (two 1s per column, sqrt(1/2) on the g=32
    chunk) -> (x_d + x_f) in PSUM; Square on ACT (2/3) or DVE-copy+Pool-mul
    (1/3) -> gstore fp16; mains: 16 accumulating matmuls + aug chunk
    [x; x^2] with coeffs [-2v; -r+2*diag(P)]; q3 via per-partition Exp bias.
Host does layout-only prep + tiny linear-term prep (w,v,q3,r,Pdd: ~3M MACs
= 0.02% of model FLOPs).
"""

import sys
from contextlib import ExitStack

import numpy as np

sys.path.insert(0, "/opt/trn_rl_repo")

import concourse.bass as bass  # noqa: E402
import concourse.tile as tile  # noqa: E402
from concourse import bacc, mybir  # noqa: E402
from concourse import bass_utils  # noqa: E402
from concourse._compat import with_exitstack  # noqa: E402
from concourse.ap import AP as RawAP  # noqa: E402

B, D, O, NCORES = 8192, 64, 256, 8
BSH = B // NCORES  # 1024 per-core batch shard
NCH = 16  # rotation-packed chunks: g = 2c+1+gj, rows (gj, d)
BT = 512  # b-tile (one PSUM bank of fp32)
NBT = BSH // BT  # 2
F32 = mybir.dt.float32
F16 = mybir.dt.float16
COPY = mybir.ActivationFunctionType.Copy
SQUARE = mybir.ActivationFunctionType.Square
EXP = mybir.ActivationFunctionType.Exp


@with_exitstack
def _kernel(ctx: ExitStack, tc, outT, xT, bt2h0, bt2h1, indc2, waug1, q3b):
    nc = tc.nc

    cpool = ctx.enter_context(tc.tile_pool(name="const", bufs=1))
    ppool = ctx.enter_context(tc.tile_pool(name="psum_p", bufs=3, space="PSUM"))
    qpool = ctx.enter_context(tc.tile_pool(name="psum_q", bufs=2, space="PSUM"))
    dpool = ctx.enter_context(tc.tile_pool(name="dram", bufs=1, space="DRAM"))

    # ---- SBUF residents ----
    warm = cpool.tile([D, BT], F16)  # warmup scratch (memset, no DMA dep)
    sb_bt2 = cpool.tile([128, 64 * 128], F16)  # [(h,e), (ttl, oo, d)]
    sb_indc2 = cpool.tile([D, NCH * 128], F16)  # two-ones rotation indicator
    aug1 = cpool.tile([128, BSH], F16)  # [xT; x^2]
    sb_waug1 = cpool.tile([128, O], F16)  # [-2 v^T; (-r+2Pdd)^T]
    sb_q3b = cpool.tile([128, 2], F32)  # -q3 per (o-half) column
    p_sb2 = cpool.tile([128, 2 * D * 64], F16)  # [(q,d), (h, f, tt)]
    w = [cpool.tile([128, NCH * 128], F16, name=f"w{h}") for h in range(2)]
    gstore = cpool.tile([128, NCH * NBT * BT], F16)  # squared features
    osb = [cpool.tile([128, BSH], F16, name=f"osb{h}") for h in range(2)]
    p_dram = dpool.tile([2, 2, D, 128, 64], F16)  # [h, q, d, f', tt]

    # ---- parallel input DMA ----
    # x and the indicator head the HWDGE queues (builds interleave into the
    # Gram stream early), then betasT2 quarters interleaved across both
    nc.sync.dma_start(aug1[0:D, :], xT[:])
    nc.scalar.dma_start(sb_indc2[:], indc2[:])
    for qx, eng in enumerate([nc.sync, nc.scalar, nc.sync, nc.scalar]):
        sl = slice(qx * 2048, (qx + 1) * 2048)
        eng.dma_start(sb_bt2[0:64, sl], bt2h0[:, sl])
    for qx, eng in enumerate([nc.sync, nc.scalar, nc.sync, nc.scalar]):
        sl = slice(qx * 2048, (qx + 1) * 2048)
        eng.dma_start(sb_bt2[64:128, sl], bt2h1[:, sl])
    nc.vector.memset(warm[:], 0.125)
    # gpsimd (SWDGE) queue: small late-needed inputs only
    nc.gpsimd.dma_start(sb_waug1[:], waug1[:])
    nc.gpsimd.dma_start(sb_q3b[:], q3b[:])

    # PSUM: ppool = 3 x [128, 1024] units (Gram + build tiles, 6 banks),
    # qpool = 2 banks (oh1 accumulators recycle oh0's after the early exp)
    pq = {}
    for oh in range(2):
        for bt in range(NBT):
            pq[(oh, bt)] = qpool.tile(
                [128, BT], F32, name=f"pq_{oh}_{bt}", tag="pq"
            )

    # ---- PE warmup: p-state ramp while input DMAs fly ----
    wps = ppool.tile([128, NBT * BT], F32, name="wps", tag="u")
    for i in range(24):
        nc.tensor.matmul(
            wps[:, 0:128], warm[:, 0:128], warm[:, 0:128], start=True, stop=True
        )

    # ---- x^2 rows of aug chunk (ACT; after xT lands) ----
    for bt in range(NBT):
        nc.scalar.activation(
            aug1[64:128, bt * BT : (bt + 1) * BT],
            aug1[0:64, bt * BT : (bt + 1) * BT],
            SQUARE,
        )

    # ---- Gram with builds interleaved ----
    # Gram pair ttl of half h covers o = h*128 + {ttl, 64+ttl}; PSUM
    # partitions (q, d); 8 pairs per 2-bank unit tile; lo block copied on
    # DVE, hi on ACT in parallel, contiguous-dst iteration.  One build
    # chunk (+square) follows every Gram unit — the betasT2 stream paces
    # Gram slower than the PE, so builds fill the idle slots for free.
    # The P round trip overlaps the copy stream via tt-split writes.
    stg = [cpool.tile([128, NBT * BT], F16, name=f"stg{i}") for i in range(2)]

    def build_chunk(c):
        bd = ppool.tile([128, NBT * BT], F32, name=f"bd_{c}", tag="u")
        for bt in range(NBT):
            nc.tensor.matmul(
                bd[:, bt * BT : (bt + 1) * BT],
                sb_indc2[:, c * 128 : (c + 1) * 128],
                aug1[0:D, bt * BT : (bt + 1) * BT],
                start=True,
                stop=True,
            )
        gsl = gstore[:, c * NBT * BT : (c + 1) * NBT * BT]
        if c % 4 != 3:
            nc.scalar.activation(gsl, bd[:], SQUARE)
        else:
            st = stg[(c // 4) % 2]
            nc.vector.tensor_copy(st[:], bd[:])
            nc.gpsimd.tensor_mul(gsl, st[:], st[:])

    pv = p_dram[:].rearrange("hh q d fp tt -> hh (q d) fp tt")
    for h in range(2):
        deng = nc.sync if h == 0 else nc.scalar
        dv = p_sb2[:, h * 4096 : (h + 1) * 4096].rearrange(
            "p (f tt) -> p f tt", f=64
        )
        for u in range(8):
            pt = ppool.tile(
                [128, NBT * BT], F32, name=f"gm_{h}_{u}", tag="u"
            )
            for t in range(8):
                ttl = u * 8 + t
                bsl = sb_bt2[h * 64 : h * 64 + 64, ttl * 128 : ttl * 128 + 128]
                nc.tensor.matmul(
                    pt[:, t * 128 : (t + 1) * 128], bsl, bsl, start=True, stop=True
                )
            sv = pt[:].rearrange("p (t oo f) -> p oo f t", t=8, oo=2)
            ts0 = u * 8
            nc.vector.tensor_copy(dv[0:64, :, ts0 : ts0 + 8], sv[0:64, 0])
            nc.scalar.activation(
                dv[64:128, :, ts0 : ts0 + 8], sv[64:128, 1], COPY
            )
            i = h * 8 + u
            if i >= 1:
                build_chunk(i - 1)  # c15 is emitted after the Gram loop
        # ---- DRAM round trip for this half (h0 on sync, h1 on scalar):
        # P at [h,q,d,f',tt] plus an f'=f+64 image of f<32 so the
        # diagonal read never wraps (full-tt writes: 128B runs)
        deng.dma_start(pv[h][:, 0:64, :], dv[:, :, :])
        deng.dma_start(pv[h][:, 64:96, :], dv[:, 0:32, :])
        # rotated W read: value P[d, (d+g)%64, o] at fp = d+g; manual AP
        # with diagonal d-stride 8256 = 8192 + 64
        base = p_dram[:]
        wv = w[h][:].rearrange("p (c o) -> p c o", c=NCH)
        for gj in range(2):
            for q in range(2):
                off = h * 1048576 + q * 524288 + (1 + gj) * 64
                srcd = RawAP(
                    tensor=base.tensor,
                    offset=base.offset + off,
                    ap=[[8256, 64], [128, NCH], [1, 64]],
                )
                deng.dma_start(
                    wv[gj * 64 : (gj + 1) * 64, :, q * 64 : (q + 1) * 64],
                    srcd,
                )

    build_chunk(NCH - 1)

    # ---- main phase A: oh0 mains ----
    for c in range(NCH):
        for bt in range(NBT):
            nc.tensor.matmul(
                pq[(0, bt)],
                w[0][:, c * 128 : (c + 1) * 128],
                gstore[:, (c * NBT + bt) * BT : (c * NBT + bt + 1) * BT],
                start=(c == 0),
                stop=False,
            )
    # aug mains + epilogue for oh0 (early, overlaps phase B)
    for bt in range(NBT):
        nc.tensor.matmul(
            pq[(0, bt)],
            sb_waug1[:, 0:128],
            aug1[:, bt * BT : (bt + 1) * BT],
            start=False,
            stop=True,
        )
        nc.scalar.activation(
            osb[0][:, bt * BT : (bt + 1) * BT],
            pq[(0, bt)],
            EXP,
            bias=sb_q3b[:, 0:1],
            scale=-1.0,
        )
        nc.sync.dma_start(
            outT[0:128, bt * BT : (bt + 1) * BT],
            osb[0][:, bt * BT : (bt + 1) * BT],
        )

    # ---- main phase B: oh1 mains from stored squares ----
    for c in range(NCH):
        for bt in range(NBT):
            nc.tensor.matmul(
                pq[(1, bt)],
                w[1][:, c * 128 : (c + 1) * 128],
                gstore[:, (c * NBT + bt) * BT : (c * NBT + bt + 1) * BT],
                start=(c == 0),
                stop=False,
            )
    for bt in range(NBT):
        nc.tensor.matmul(
            pq[(1, bt)],
            sb_waug1[:, 128:256],
            aug1[:, bt * BT : (bt + 1) * BT],
            start=False,
            stop=True,
        )
        nc.scalar.activation(
            osb[1][:, bt * BT : (bt + 1) * BT],
            pq[(1, bt)],
            EXP,
            bias=sb_q3b[:, 1:2],
            scale=-1.0,
        )
        nc.scalar.dma_start(
            outT[128:256, bt * BT : (bt + 1) * BT],
            osb[1][:, bt * BT : (bt + 1) * BT],
        )


_CACHE = {}


def _build():
    if "nc" in _CACHE:
        return _CACHE["nc"], _CACHE["aps"]
    nc = bacc.Bacc(
        "TRN2", target_bir_lowering=False, debug=False, num_devices=NCORES
    )
    xT = nc.dram_tensor("xT", [D, BSH], F16, kind="ExternalInput").ap()
    bt2h0 = nc.dram_tensor("bt2h0", [64, 8192], F16, kind="ExternalInput").ap()
    bt2h1 = nc.dram_tensor("bt2h1", [64, 8192], F16, kind="ExternalInput").ap()
    indc2 = nc.dram_tensor("indc2", [D, NCH * 128], F16, kind="ExternalInput").ap()
    waug1 = nc.dram_tensor("waug1", [128, O], F16, kind="ExternalInput").ap()
    q3b = nc.dram_tensor("q3b", [128, 2], F32, kind="ExternalInput").ap()
    outT = nc.dram_tensor("outT", [O, BSH], F16, kind="ExternalOutput").ap()
    with tile.TileContext(nc) as tc:
        _kernel(tc, outT, xT, bt2h0, bt2h1, indc2, waug1, q3b)
    nc.compile()
    _CACHE["nc"] = nc
    _CACHE["aps"] = (xT, bt2h0, bt2h1, indc2, waug1, q3b, outT)
    return nc, _CACHE["aps"]


def _host_prep(x, centers, betas):
    x = np.asarray(x, np.float32)
    betas = np.asarray(betas, np.float32)
    c = np.asarray(centers, np.float32).reshape(O, D)
    # betasT2[h][e, (ttl, oo, d)] = betas[h*128+oo*64+ttl, d, e]  (unscaled:
    # Gram produces P exactly, the pair coefficient A/2 = P for g<32)
    bt = betas.transpose(2, 0, 1)  # [e, o, d]
    bt = bt.reshape(D, 2, 2, 64, D)  # [e, h, oo, ttl, d]
    bt = bt.transpose(1, 0, 3, 2, 4).reshape(2, D, 8192)  # [h, e, (ttl,oo,d)]
    bt2h0 = np.ascontiguousarray(bt[0]).astype(np.float16)
    bt2h1 = np.ascontiguousarray(bt[1]).astype(np.float16)
    # rotation indicator: chunk c rows p=(gj,d), g=2c+1+gj:
    # indc2[k, c*128+p] = s * ([k == d] + [k == (d+g)%64]), s = sqrt(1/2)
    # only for the g=32 rows (each {d, d+32} pair is covered twice)
    k = np.arange(D)[:, None, None, None]
    cc = np.arange(NCH)[None, :, None, None]
    gj = np.arange(2)[None, None, :, None]
    dd = np.arange(D)[None, None, None, :]
    g = 2 * cc + 1 + gj
    ind = (k == dd).astype(np.float32) + (k == (dd + g) % D)
    ind *= np.where(g == 32, np.sqrt(0.5), 1.0)
    indc2 = np.ascontiguousarray(ind.reshape(D, NCH * 128)).astype(np.float16)
    # tiny linear-term prep: w = B^T c, v = B w, q3 = w.w, r = P 1,
    # Pdd = diag(P)  (~3M MACs total)
    w_ = np.einsum("ofe,of->oe", betas, c)
    v = np.einsum("ode,oe->od", betas, w_)
    q3 = np.einsum("oe,oe->o", w_, w_)
    s = betas.sum(axis=1)  # [o, e]
    r = np.einsum("ode,oe->od", betas, s)
    pdd = np.einsum("ode,ode->od", betas, betas)
    waug1 = np.concatenate([-2.0 * v.T, (-r + 2.0 * pdd).T], axis=0).astype(
        np.float16
    )
    q3b = np.ascontiguousarray((-q3).reshape(2, 128).T).astype(np.float32)
    xT_shards = [
        np.ascontiguousarray(x[i * BSH : (i + 1) * BSH].T).astype(np.float16)
        for i in range(NCORES)
    ]
    return xT_shards, bt2h0, bt2h1, indc2, waug1, q3b


def _run(x, centers, betas, trace=False):
    nc, (xT, bt2h0a, bt2h1a, indc2a, waug1a, q3ba, outT) = _build()
    xT_shards, bt2h0, bt2h1, indc2, waug1, q3b = _host_prep(x, centers, betas)
    in_maps = [
        {
            xT.name: xT_shards[i],
            bt2h0a.name: bt2h0,
            bt2h1a.name: bt2h1,
            indc2a.name: indc2,
            waug1a.name: waug1,
            q3ba.name: q3b,
        }
        for i in range(NCORES)
    ]
    res = bass_utils.run_bass_kernel_spmd(
        nc, in_maps, core_ids=list(range(NCORES)), trace=trace
    )
    out = np.concatenate(
        [np.asarray(res.results[i][outT.name]).T for i in range(NCORES)],
        axis=0,
    )
    return out.astype(np.float32), res


def kernel(x, centers, betas):
    out, _ = _run(x, centers, betas, trace=False)
    return out


# revision 34
# speedup vs baseline: 1.5988x; 1.0862x over previous
"""Trainium2 Bass kernel for nn_Cov_EBFLayer.

Math: out[b,o] = exp(-quad[o,b]),
  quad[o,b] = diff^T P_o diff,  diff = c_o - x_b,  P_o = B_o B_o^T
            = x^T P x - 2 v_o^T x + q3_o,   v = P c,  q3 = c^T P c
Square trick + rotation packing:
  x^T P x = sum_{d, g=1..32} s2_g * P[d, (d+g)%64] * (x_d + x_{(d+g)%64})^2
            - sum_d (r_d - 2 P_dd) x_d^2
  with s2_g = 1 for g<32, 1/2 for g=32 (each unordered pair covered once by
  g=1..31, twice by g=32), r = P 1.  The 2080 unique pair features pack into
  exactly 16 chunks of 128 rows (gj,d), g = 2c+1+gj — HALF the matmuls of
  the naive 32-chunk full-square map.

Kernel per core (batch-sharded 8 x 1024):
  - warmup matmuls from a memset tile (PE p-state ramp, no DMA dependency)
  - Gram: P_o = B_o^T B_o for o-pairs (ol, ol+64), PSUM partitions (q, d),
    copies (DVE lo / ACT hi in parallel) -> p_sb2 [(q,d), (h,f,tt)]
  - DRAM round trip per o-half: write P at [h,q,d,f',tt] with an extra
    f'=f+64 image of f<32 so the rotated read never wraps; re-read W chunks
    [(gj,d), (c, o)] with a manual diagonal AP (d-stride 8256 = (128+1)*64).
  - builds: indicator matmul (two 1s per column, sqrt(1/2) on the g=32
    chunk) -> (x_d + x_f) in PSUM; Square on ACT (2/3) or DVE-copy+Pool-mul
    (1/3) -> gstore fp16; mains: 16 accumulating matmuls + aug chunk
    [x; x^2] with coeffs [-2v; -r+2*diag(P)]; q3 via per-partition Exp bias.
Host does layout-only prep + tiny linear-term prep (w,v,q3,r,Pdd: ~3M MACs
= 0.02% of model FLOPs).
"""

import sys
from contextlib import ExitStack

import numpy as np

sys.path.insert(0, "/opt/trn_rl_repo")

import concourse.bass as bass  # noqa: E402
import concourse.tile as tile  # noqa: E402
from concourse import bacc, mybir  # noqa: E402
from concourse import bass_utils  # noqa: E402
from concourse._compat import with_exitstack  # noqa: E402
from concourse.ap import AP as RawAP  # noqa: E402

B, D, O, NCORES = 8192, 64, 256, 8
BSH = B // NCORES  # 1024 per-core batch shard
NCH = 16  # rotation-packed chunks: g = 2c+1+gj, rows (gj, d)
BT = 512  # b-tile (one PSUM bank of fp32)
NBT = BSH // BT  # 2
F32 = mybir.dt.float32
F16 = mybir.dt.float16
COPY = mybir.ActivationFunctionType.Copy
SQUARE = mybir.ActivationFunctionType.Square
EXP = mybir.ActivationFunctionType.Exp


@with_exitstack
def _kernel(ctx: ExitStack, tc, outT, xT, bt2h0, bt2h1, indc2, waug1, q3b):
    nc = tc.nc

    cpool = ctx.enter_context(tc.tile_pool(name="const", bufs=1))
    ppool = ctx.enter_context(tc.tile_pool(name="psum_p", bufs=3, space="PSUM"))
    qpool = ctx.enter_context(tc.tile_pool(name="psum_q", bufs=2, space="PSUM"))
    dpool = ctx.enter_context(tc.tile_pool(name="dram", bufs=1, space="DRAM"))

    # ---- SBUF residents ----
    warm = cpool.tile([D, BT], F16)  # warmup scratch (memset, no DMA dep)
    sb_bt2 = cpool.tile([128, 64 * 128], F16)  # [(h,e), (ttl, oo, d)]
    sb_indc2 = cpool.tile([D, NCH * 128], F16)  # two-ones rotation indicator
    aug1 = cpool.tile([128, BSH], F16)  # [xT; x^2]
    sb_waug1 = cpool.tile([128, O], F16)  # [-2 v^T; (-r+2Pdd)^T]
    sb_q3b = cpool.tile([128, 2], F32)  # -q3 per (o-half) column
    p_sb2 = cpool.tile([128, 2 * D * 64], F16)  # [(q,d), (h, f, tt)]
    w = [cpool.tile([128, NCH * 128], F16, name=f"w{h}") for h in range(2)]
    gstore = cpool.tile([128, NCH * NBT * BT], F16)  # squared features
    osb = [cpool.tile([128, BSH], F16, name=f"osb{h}") for h in range(2)]
    p_dram = dpool.tile([2, 2, D, 128, 64], F16)  # [h, q, d, f', tt]

    # ---- parallel input DMA ----
    # betasT2 quarters interleaved across sync+scalar (half 0 first)
    for qx, eng in enumerate([nc.sync, nc.scalar, nc.sync, nc.scalar]):
        sl = slice(qx * 2048, (qx + 1) * 2048)
        eng.dma_start(sb_bt2[0:64, sl], bt2h0[:, sl])
    for qx, eng in enumerate([nc.sync, nc.scalar, nc.sync, nc.scalar]):
        sl = slice(qx * 2048, (qx + 1) * 2048)
        eng.dma_start(sb_bt2[64:128, sl], bt2h1[:, sl])
    nc.vector.memset(warm[:], 0.125)
    # gpsimd (SWDGE) queue: small inputs (needed only after the Gram phase)
    nc.gpsimd.dma_start(aug1[0:D, :], xT[:])
    nc.gpsimd.dma_start(sb_indc2[:], indc2[:])
    nc.gpsimd.dma_start(sb_waug1[:], waug1[:])
    nc.gpsimd.dma_start(sb_q3b[:], q3b[:])

    # PSUM: ppool = 3 x [128, 1024] units (Gram + build tiles, 6 banks),
    # qpool = 2 banks (oh1 accumulators recycle oh0's after the early exp)
    pq = {}
    for oh in range(2):
        for bt in range(NBT):
            pq[(oh, bt)] = qpool.tile(
                [128, BT], F32, name=f"pq_{oh}_{bt}", tag="pq"
            )

    # ---- PE warmup: p-state ramp while input DMAs fly ----
    wps = ppool.tile([128, NBT * BT], F32, name="wps", tag="u")
    for i in range(7):
        nc.tensor.matmul(
            wps[:, 0:BT], warm[:, 0:128], warm[:], start=True, stop=True
        )

    # ---- x^2 rows of aug chunk (ACT; after xT lands) ----
    for bt in range(NBT):
        nc.scalar.activation(
            aug1[64:128, bt * BT : (bt + 1) * BT],
            aug1[0:64, bt * BT : (bt + 1) * BT],
            SQUARE,
        )

    # ---- Gram with builds interleaved ----
    # Gram pair ttl of half h covers o = h*128 + {ttl, 64+ttl}; PSUM
    # partitions (q, d); 8 pairs per 2-bank unit tile; lo block copied on
    # DVE, hi on ACT in parallel, contiguous-dst iteration.  One build
    # chunk (+square) follows every Gram unit — the betasT2 stream paces
    # Gram slower than the PE, so builds fill the idle slots for free.
    # The P round trip overlaps the copy stream via tt-split writes.
    stg = [cpool.tile([128, NBT * BT], F16, name=f"stg{i}") for i in range(2)]

    def build_chunk(c):
        bd = ppool.tile([128, NBT * BT], F32, name=f"bd_{c}", tag="u")
        for bt in range(NBT):
            nc.tensor.matmul(
                bd[:, bt * BT : (bt + 1) * BT],
                sb_indc2[:, c * 128 : (c + 1) * 128],
                aug1[0:D, bt * BT : (bt + 1) * BT],
                start=True,
                stop=True,
            )
        gsl = gstore[:, c * NBT * BT : (c + 1) * NBT * BT]
        if c % 4 != 3:
            nc.scalar.activation(gsl, bd[:], SQUARE)
        else:
            st = stg[(c // 4) % 2]
            nc.vector.tensor_copy(st[:], bd[:])
            nc.gpsimd.tensor_mul(gsl, st[:], st[:])

    pv = p_dram[:].rearrange("hh q d fp tt -> hh (q d) fp tt")
    for h in range(2):
        deng = nc.sync if h == 0 else nc.scalar
        dv = p_sb2[:, h * 4096 : (h + 1) * 4096].rearrange(
            "p (f tt) -> p f tt", f=64
        )
        for u in range(8):
            pt = ppool.tile(
                [128, NBT * BT], F32, name=f"gm_{h}_{u}", tag="u"
            )
            for t in range(8):
                ttl = u * 8 + t
                bsl = sb_bt2[h * 64 : h * 64 + 64, ttl * 128 : ttl * 128 + 128]
                nc.tensor.matmul(
                    pt[:, t * 128 : (t + 1) * 128], bsl, bsl, start=True, stop=True
                )
            sv = pt[:].rearrange("p (t oo f) -> p oo f t", t=8, oo=2)
            ts0 = u * 8
            nc.vector.tensor_copy(dv[0:64, :, ts0 : ts0 + 8], sv[0:64, 0])
            nc.scalar.activation(
                dv[64:128, :, ts0 : ts0 + 8], sv[64:128, 1], COPY
            )
        # ---- DRAM round trip for this half (h0 on sync, h1 on scalar):
        # P at [h,q,d,f',tt] plus an f'=f+64 image of f<32 so the
        # diagonal read never wraps (full-tt writes: 128B runs)
        deng.dma_start(pv[h][:, 0:64, :], dv[:, :, :])
        deng.dma_start(pv[h][:, 64:96, :], dv[:, 0:32, :])
        # rotated W read: value P[d, (d+g)%64, o] at fp = d+g; manual AP
        # with diagonal d-stride 8256 = 8192 + 64
        base = p_dram[:]
        wv = w[h][:].rearrange("p (c o) -> p c o", c=NCH)
        for gj in range(2):
            for q in range(2):
                off = h * 1048576 + q * 524288 + (1 + gj) * 64
                srcd = RawAP(
                    tensor=base.tensor,
                    offset=base.offset + off,
                    ap=[[8256, 64], [128, NCH], [1, 64]],
                )
                deng.dma_start(
                    wv[gj * 64 : (gj + 1) * 64, :, q * 64 : (q + 1) * 64],
                    srcd,
                )

    # ---- main phase A: builds + squares + oh0 mains (mains trail builds
    # so the PE queue never head-blocks on the w[0] round trip) ----
    TRAIL = 12
    for cc in range(NCH + TRAIL):
        if cc < NCH:
            build_chunk(cc)
        if cc >= TRAIL:
            c = cc - TRAIL
            for bt in range(NBT):
                nc.tensor.matmul(
                    pq[(0, bt)],
                    w[0][:, c * 128 : (c + 1) * 128],
                    gstore[:, (c * NBT + bt) * BT : (c * NBT + bt + 1) * BT],
                    start=(c == 0),
                    stop=False,
                )
    # aug mains + epilogue for oh0 (early, overlaps phase B)
    for bt in range(NBT):
        nc.tensor.matmul(
            pq[(0, bt)],
            sb_waug1[:, 0:128],
            aug1[:, bt * BT : (bt + 1) * BT],
            start=False,
            stop=True,
        )
        nc.scalar.activation(
            osb[0][:, bt * BT : (bt + 1) * BT],
            pq[(0, bt)],
            EXP,
            bias=sb_q3b[:, 0:1],
            scale=-1.0,
        )
        nc.sync.dma_start(
            outT[0:128, bt * BT : (bt + 1) * BT],
            osb[0][:, bt * BT : (bt + 1) * BT],
        )

    # ---- main phase B: oh1 mains from stored squares ----
    for c in range(NCH):
        for bt in range(NBT):
            nc.tensor.matmul(
                pq[(1, bt)],
                w[1][:, c * 128 : (c + 1) * 128],
                gstore[:, (c * NBT + bt) * BT : (c * NBT + bt + 1) * BT],
                start=(c == 0),
                stop=False,
            )
    for bt in range(NBT):
        nc.tensor.matmul(
            pq[(1, bt)],
            sb_waug1[:, 128:256],
            aug1[:, bt * BT : (bt + 1) * BT],
            start=False,
            stop=True,
        )
        nc.scalar.activation(
            osb[1][:, bt * BT : (bt + 1) * BT],
            pq[(1, bt)],
            EXP,
            bias=sb_q3b[:, 1:2],
            scale=-1.0,
        )
        nc.scalar.dma_start(
            outT[128:256, bt * BT : (bt + 1) * BT],
            osb[1][:, bt * BT : (bt + 1) * BT],
        )


_CACHE = {}


def _build():
    if "nc" in _CACHE:
        return _CACHE["nc"], _CACHE["aps"]
    nc = bacc.Bacc(
        "TRN2", target_bir_lowering=False, debug=False, num_devices=NCORES
    )
    xT = nc.dram_tensor("xT", [D, BSH], F16, kind="ExternalInput").ap()
    bt2h0 = nc.dram_tensor("bt2h0", [64, 8192], F16, kind="ExternalInput").ap()
    bt2h1 = nc.dram_tensor("bt2h1", [64, 8192], F16, kind="ExternalInput").ap()
    indc2 = nc.dram_tensor("indc2", [D, NCH * 128], F16, kind="ExternalInput").ap()
    waug1 = nc.dram_tensor("waug1", [128, O], F16, kind="ExternalInput").ap()
    q3b = nc.dram_tensor("q3b", [128, 2], F32, kind="ExternalInput").ap()
    outT = nc.dram_tensor("outT", [O, BSH], F16, kind="ExternalOutput").ap()
    with tile.TileContext(nc) as tc:
        _kernel(tc, outT, xT, bt2h0, bt2h1, indc2, waug1, q3b)
    nc.compile()
    _CACHE["nc"] = nc
    _CACHE["aps"] = (xT, bt2h0, bt2h1, indc2, waug1, q3b, outT)
    return nc, _CACHE["aps"]


def _host_prep(x, centers, betas):
    x = np.asarray(x, np.float32)
    betas = np.asarray(betas, np.float32)
    c = np.asarray(centers, np.float32).reshape(O, D)
    # betasT2[h][e, (ttl, oo, d)] = betas[h*128+oo*64+ttl, d, e]  (unscaled:
    # Gram produces P exactly, the pair coefficient A/2 = P for g<32)
    bt = betas.transpose(2, 0, 1)  # [e, o, d]
    bt = bt.reshape(D, 2, 2, 64, D)  # [e, h, oo, ttl, d]
    bt = bt.transpose(1, 0, 3, 2, 4).reshape(2, D, 8192)  # [h, e, (ttl,oo,d)]
    bt2h0 = np.ascontiguousarray(bt[0]).astype(np.float16)
    bt2h1 = np.ascontiguousarray(bt[1]).astype(np.float16)
    # rotation indicator: chunk c rows p=(gj,d), g=2c+1+gj:
    # indc2[k, c*128+p] = s * ([k == d] + [k == (d+g)%64]), s = sqrt(1/2)
    # only for the g=32 rows (each {d, d+32} pair is covered twice)
    k = np.arange(D)[:, None, None, None]
    cc = np.arange(NCH)[None, :, None, None]
    gj = np.arange(2)[None, None, :, None]
    dd = np.arange(D)[None, None, None, :]
    g = 2 * cc + 1 + gj
    ind = (k == dd).astype(np.float32) + (k == (dd + g) % D)
    ind *= np.where(g == 32, np.sqrt(0.5), 1.0)
    indc2 = np.ascontiguousarray(ind.reshape(D, NCH * 128)).astype(np.float16)
    # tiny linear-term prep: w = B^T c, v = B w, q3 = w.w, r = P 1,
    # Pdd = diag(P)  (~3M MACs total)
    w_ = np.einsum("ofe,of->oe", betas, c)
    v = np.einsum("ode,oe->od", betas, w_)
    q3 = np.einsum("oe,oe->o", w_, w_)
    s = betas.sum(axis=1)  # [o, e]
    r = np.einsum("ode,oe->od", betas, s)
    pdd = np.einsum("ode,ode->od", betas, betas)
    waug1 = np.concatenate([-2.0 * v.T, (-r + 2.0 * pdd).T], axis=0).astype(
        np.float16
    )
    q3b = np.ascontiguousarray((-q3).reshape(2, 128).T).astype(np.float32)
    xT_shards = [
        np.ascontiguousarray(x[i * BSH : (i + 1) * BSH].T).astype(np.float16)
        for i in range(NCORES)
    ]
    return xT_shards, bt2h0, bt2h1, indc2, waug1, q3b


def _run(x, centers, betas, trace=False):
    nc, (xT, bt2h0a, bt2h1a, indc2a, waug1a, q3ba, outT) = _build()
    xT_shards, bt2h0, bt2h1, indc2, waug1, q3b = _host_prep(x, centers, betas)
    in_maps = [
        {
            xT.name: xT_shards[i],
            bt2h0a.name: bt2h0,
            bt2h1a.name: bt2h1,
            indc2a.name: indc2,
            waug1a.name: waug1,
            q3ba.name: q3b,
        }
        for i in range(NCORES)
    ]
    res = bass_utils.run_bass_kernel_spmd(
        nc, in_maps, core_ids=list(range(NCORES)), trace=trace
    )
    out = np.concatenate(
        [np.asarray(res.results[i][outT.name]).T for i in range(NCORES)],
        axis=0,
    )
    return out.astype(np.float32), res


def kernel(x, centers, betas):
    out, _ = _run(x, centers, betas, trace=False)
    return out


# revision 36
# speedup vs baseline: 1.6464x; 1.0298x over previous
"""Trainium2 Bass kernel for nn_Cov_EBFLayer.

Math: out[b,o] = exp(-quad[o,b]),
  quad[o,b] = diff^T P_o diff,  diff = c_o - x_b,  P_o = B_o B_o^T
            = x^T P x - 2 v_o^T x + q3_o,   v = P c,  q3 = c^T P c
Square trick + rotation packing:
  x^T P x = sum_{d, g=1..32} s2_g * P[d, (d+g)%64] * (x_d + x_{(d+g)%64})^2
            - sum_d (r_d - 2 P_dd) x_d^2
  with s2_g = 1 for g<32, 1/2 for g=32 (each unordered pair covered once by
  g=1..31, twice by g=32), r = P 1.  The 2080 unique pair features pack into
  exactly 16 chunks of 128 rows (gj,d), g = 2c+1+gj — HALF the matmuls of
  the naive 32-chunk full-square map.

Kernel per core (batch-sharded 8 x 1024):
  - warmup matmuls from a memset tile (PE p-state ramp, no DMA dependency)
  - Gram: P_o = B_o^T B_o for o-pairs (ol, ol+64), PSUM partitions (q, d),
    copies (DVE lo / ACT hi in parallel) -> p_sb2 [(q,d), (h,f,tt)]
  - DRAM round trip per o-half: write P at [h,q,d,f',tt] with an extra
    f'=f+64 image of f<32 so the rotated read never wraps; re-read W chunks
    [(gj,d), (c, o)] with a manual diagonal AP (d-stride 8256 = (128+1)*64).
  - builds: indicator matmul (two 1s per column, sqrt(1/2) on the g=32
    chunk) -> (x_d + x_f) in PSUM; Square on ACT (2/3) or DVE-copy+Pool-mul
    (1/3) -> gstore fp16; mains: 16 accumulating matmuls + aug chunk
    [x; x^2] with coeffs [-2v; -r+2*diag(P)]; q3 via per-partition Exp bias.
Host does layout-only prep + tiny linear-term prep (w,v,q3,r,Pdd: ~3M MACs
= 0.02% of model FLOPs).
"""

import sys
from contextlib import ExitStack

import numpy as np

sys.path.insert(0, "/opt/trn_rl_repo")

import concourse.bass as bass  # noqa: E402
import concourse.tile as tile  # noqa: E402
from concourse import bacc, mybir  # noqa: E402
from concourse import bass_utils  # noqa: E402
from concourse._compat import with_exitstack  # noqa: E402
from concourse.ap import AP as RawAP  # noqa: E402

B, D, O, NCORES = 8192, 64, 256, 8
BSH = B // NCORES  # 1024 per-core batch shard
NCH = 16  # rotation-packed chunks: g = 2c+1+gj, rows (gj, d)
BT = 512  # b-tile (one PSUM bank of fp32)
NBT = BSH // BT  # 2
F32 = mybir.dt.float32
F16 = mybir.dt.float16
COPY = mybir.ActivationFunctionType.Copy
SQUARE = mybir.ActivationFunctionType.Square
EXP = mybir.ActivationFunctionType.Exp


@with_exitstack
def _kernel(ctx: ExitStack, tc, outT, xT, bt2h0, bt2h1, indc2, waug1, q3b):
    nc = tc.nc

    cpool = ctx.enter_context(tc.tile_pool(name="const", bufs=1))
    ppool = ctx.enter_context(tc.tile_pool(name="psum_p", bufs=3, space="PSUM"))
    qpool = ctx.enter_context(tc.tile_pool(name="psum_q", bufs=2, space="PSUM"))
    dpool = ctx.enter_context(tc.tile_pool(name="dram", bufs=1, space="DRAM"))

    # ---- SBUF residents ----
    warm = cpool.tile([D, BT], F16)  # warmup scratch (memset, no DMA dep)
    sb_bt2 = cpool.tile([128, 64 * 128], F16)  # [(h,e), (ttl, oo, d)]
    sb_indc2 = cpool.tile([D, NCH * 128], F16)  # two-ones rotation indicator
    aug1 = cpool.tile([128, BSH], F16)  # [xT; x^2]
    sb_waug1 = cpool.tile([128, O], F16)  # [-2 v^T; (-r+2Pdd)^T]
    sb_q3b = cpool.tile([128, 2], F32)  # -q3 per (o-half) column
    p_sb2 = cpool.tile([128, 2 * D * 64], F16)  # [(q,d), (h, f, tt)]
    w = [cpool.tile([128, NCH * 128], F16, name=f"w{h}") for h in range(2)]
    gstore = cpool.tile([128, NCH * NBT * BT], F16)  # squared features
    osb = [cpool.tile([128, BSH], F16, name=f"osb{h}") for h in range(2)]
    p_dram = dpool.tile([2, 2, D, 128, 64], F16)  # [h, q, d, f', tt]

    # ---- parallel input DMA ----
    # betasT2 quarters interleaved across sync+scalar (half 0 first)
    for qx, eng in enumerate([nc.sync, nc.scalar, nc.sync, nc.scalar]):
        sl = slice(qx * 2048, (qx + 1) * 2048)
        eng.dma_start(sb_bt2[0:64, sl], bt2h0[:, sl])
    for qx, eng in enumerate([nc.sync, nc.scalar, nc.sync, nc.scalar]):
        sl = slice(qx * 2048, (qx + 1) * 2048)
        eng.dma_start(sb_bt2[64:128, sl], bt2h1[:, sl])
    nc.vector.memset(warm[:], 0.125)
    # gpsimd (SWDGE) queue: small inputs (needed only after the Gram phase)
    nc.gpsimd.dma_start(aug1[0:D, :], xT[:])
    nc.gpsimd.dma_start(sb_indc2[:], indc2[:])
    nc.gpsimd.dma_start(sb_waug1[:], waug1[:])
    nc.gpsimd.dma_start(sb_q3b[:], q3b[:])

    # PSUM: ppool = 3 x [128, 1024] units (Gram + build tiles, 6 banks),
    # qpool = 2 banks (oh1 accumulators recycle oh0's after the early exp)
    pq = {}
    for oh in range(2):
        for bt in range(NBT):
            pq[(oh, bt)] = qpool.tile(
                [128, BT], F32, name=f"pq_{oh}_{bt}", tag="pq"
            )

    # ---- PE warmup: p-state ramp while input DMAs fly ----
    wps = ppool.tile([128, NBT * BT], F32, name="wps", tag="u")
    for i in range(7):
        nc.tensor.matmul(
            wps[:, 0:BT], warm[:, 0:128], warm[:], start=True, stop=True
        )

    # ---- x^2 rows of aug chunk (ACT; after xT lands) ----
    for bt in range(NBT):
        nc.scalar.activation(
            aug1[64:128, bt * BT : (bt + 1) * BT],
            aug1[0:64, bt * BT : (bt + 1) * BT],
            SQUARE,
        )

    # ---- Gram with builds interleaved ----
    # Gram pair ttl of half h covers o = h*128 + {ttl, 64+ttl}; PSUM
    # partitions (q, d); 8 pairs per 2-bank unit tile; lo block copied on
    # DVE, hi on ACT in parallel, contiguous-dst iteration.  One build
    # chunk (+square) follows every Gram unit — the betasT2 stream paces
    # Gram slower than the PE, so builds fill the idle slots for free.
    # The P round trip overlaps the copy stream via tt-split writes.
    stg = [cpool.tile([128, NBT * BT], F16, name=f"stg{i}") for i in range(2)]

    def build_chunk(c):
        bd = ppool.tile([128, NBT * BT], F32, name=f"bd_{c}", tag="u")
        for bt in range(NBT):
            nc.tensor.matmul(
                bd[:, bt * BT : (bt + 1) * BT],
                sb_indc2[:, c * 128 : (c + 1) * 128],
                aug1[0:D, bt * BT : (bt + 1) * BT],
                start=True,
                stop=True,
            )
        gsl = gstore[:, c * NBT * BT : (c + 1) * NBT * BT]
        if c % 3 != 2:
            nc.scalar.activation(gsl, bd[:], SQUARE)
        else:
            st = stg[(c // 3) % 2]
            nc.vector.tensor_copy(st[:], bd[:])
            nc.gpsimd.tensor_mul(gsl, st[:], st[:])

    pv = p_dram[:].rearrange("hh q d fp tt -> hh (q d) fp tt")
    for h in range(2):
        deng = nc.sync if h == 0 else nc.scalar
        dv = p_sb2[:, h * 4096 : (h + 1) * 4096].rearrange(
            "p (f tt) -> p f tt", f=64
        )
        for u in range(8):
            pt = ppool.tile(
                [128, NBT * BT], F32, name=f"gm_{h}_{u}", tag="u"
            )
            for t in range(8):
                ttl = u * 8 + t
                bsl = sb_bt2[h * 64 : h * 64 + 64, ttl * 128 : ttl * 128 + 128]
                nc.tensor.matmul(
                    pt[:, t * 128 : (t + 1) * 128], bsl, bsl, start=True, stop=True
                )
            sv = pt[:].rearrange("p (t oo f) -> p oo f t", t=8, oo=2)
            ts0 = u * 8
            nc.vector.tensor_copy(dv[0:64, :, ts0 : ts0 + 8], sv[0:64, 0])
            nc.scalar.activation(
                dv[64:128, :, ts0 : ts0 + 8], sv[64:128, 1], COPY
            )
        # ---- DRAM round trip for this half (h0 on sync, h1 on scalar):
        # P at [h,q,d,f',tt] plus an f'=f+64 image of f<32 so the
        # diagonal read never wraps (full-tt writes: 128B runs)
        deng.dma_start(pv[h][:, 0:64, :], dv[:, :, :])
        deng.dma_start(pv[h][:, 64:96, :], dv[:, 0:32, :])
        # rotated W read: value P[d, (d+g)%64, o] at fp = d+g; manual AP
        # with diagonal d-stride 8256 = 8192 + 64
        base = p_dram[:]
        wv = w[h][:].rearrange("p (c o) -> p c o", c=NCH)
        for gj in range(2):
            for q in range(2):
                off = h * 1048576 + q * 524288 + (1 + gj) * 64
                srcd = RawAP(
                    tensor=base.tensor,
                    offset=base.offset + off,
                    ap=[[8256, 64], [128, NCH], [1, 64]],
                )
                deng.dma_start(
                    wv[gj * 64 : (gj + 1) * 64, :, q * 64 : (q + 1) * 64],
                    srcd,
                )

    # ---- main phase A: builds + squares + oh0 mains (mains trail builds
    # so the PE queue never head-blocks on the w[0] round trip) ----
    TRAIL = 10
    for cc in range(NCH + TRAIL):
        if cc < NCH:
            build_chunk(cc)
        if cc >= TRAIL:
            c = cc - TRAIL
            for bt in range(NBT):
                nc.tensor.matmul(
                    pq[(0, bt)],
                    w[0][:, c * 128 : (c + 1) * 128],
                    gstore[:, (c * NBT + bt) * BT : (c * NBT + bt + 1) * BT],
                    start=(c == 0),
                    stop=False,
                )
    # aug mains + epilogue for oh0 (early, overlaps phase B)
    for bt in range(NBT):
        nc.tensor.matmul(
            pq[(0, bt)],
            sb_waug1[:, 0:128],
            aug1[:, bt * BT : (bt + 1) * BT],
            start=False,
            stop=True,
        )
        nc.scalar.activation(
            osb[0][:, bt * BT : (bt + 1) * BT],
            pq[(0, bt)],
            EXP,
            bias=sb_q3b[:, 0:1],
            scale=-1.0,
        )
        nc.sync.dma_start(
            outT[0:128, bt * BT : (bt + 1) * BT],
            osb[0][:, bt * BT : (bt + 1) * BT],
        )

    # ---- main phase B: oh1 mains from stored squares ----
    for c in range(NCH):
        for bt in range(NBT):
            nc.tensor.matmul(
                pq[(1, bt)],
                w[1][:, c * 128 : (c + 1) * 128],
                gstore[:, (c * NBT + bt) * BT : (c * NBT + bt + 1) * BT],
                start=(c == 0),
                stop=False,
            )
    for bt in range(NBT):
        nc.tensor.matmul(
            pq[(1, bt)],
            sb_waug1[:, 128:256],
            aug1[:, bt * BT : (bt + 1) * BT],
            start=False,
            stop=True,
        )
        nc.scalar.activation(
            osb[1][:, bt * BT : (bt + 1) * BT],
            pq[(1, bt)],
            EXP,
            bias=sb_q3b[:, 1:2],
            scale=-1.0,
        )
        nc.scalar.dma_start(
            outT[128:256, bt * BT : (bt + 1) * BT],
            osb[1][:, bt * BT : (bt + 1) * BT],
        )


_CACHE = {}


def _build():
    if "nc" in _CACHE:
        return _CACHE["nc"], _CACHE["aps"]
    nc = bacc.Bacc(
        "TRN2", target_bir_lowering=False, debug=False, num_devices=NCORES
    )
    xT = nc.dram_tensor("xT", [D, BSH], F16, kind="ExternalInput").ap()
    bt2h0 = nc.dram_tensor("bt2h0", [64, 8192], F16, kind="ExternalInput").ap()
    bt2h1 = nc.dram_tensor("bt2h1", [64, 8192], F16, kind="ExternalInput").ap()
    indc2 = nc.dram_tensor("indc2", [D, NCH * 128], F16, kind="ExternalInput").ap()
    waug1 = nc.dram_tensor("waug1", [128, O], F16, kind="ExternalInput").ap()
    q3b = nc.dram_tensor("q3b", [128, 2], F32, kind="ExternalInput").ap()
    outT = nc.dram_tensor("outT", [O, BSH], F16, kind="ExternalOutput").ap()
    with tile.TileContext(nc) as tc:
        _kernel(tc, outT, xT, bt2h0, bt2h1, indc2, waug1, q3b)
    nc.compile()
    _CACHE["nc"] = nc
    _CACHE["aps"] = (xT, bt2h0, bt2h1, indc2, waug1, q3b, outT)
    return nc, _CACHE["aps"]


def _host_prep(x, centers, betas):
    x = np.asarray(x, np.float32)
    betas = np.asarray(betas, np.float32)
    c = np.asarray(centers, np.float32).reshape(O, D)
    # betasT2[h][e, (ttl, oo, d)] = betas[h*128+oo*64+ttl, d, e]  (unscaled:
    # Gram produces P exactly, the pair coefficient A/2 = P for g<32)
    bt = betas.transpose(2, 0, 1)  # [e, o, d]
    bt = bt.reshape(D, 2, 2, 64, D)  # [e, h, oo, ttl, d]
    bt = bt.transpose(1, 0, 3, 2, 4).reshape(2, D, 8192)  # [h, e, (ttl,oo,d)]
    bt2h0 = np.ascontiguousarray(bt[0]).astype(np.float16)
    bt2h1 = np.ascontiguousarray(bt[1]).astype(np.float16)
    # rotation indicator: chunk c rows p=(gj,d), g=2c+1+gj:
    # indc2[k, c*128+p] = s * ([k == d] + [k == (d+g)%64]), s = sqrt(1/2)
    # only for the g=32 rows (each {d, d+32} pair is covered twice)
    k = np.arange(D)[:, None, None, None]
    cc = np.arange(NCH)[None, :, None, None]
    gj = np.arange(2)[None, None, :, None]
    dd = np.arange(D)[None, None, None, :]
    g = 2 * cc + 1 + gj
    ind = (k == dd).astype(np.float32) + (k == (dd + g) % D)
    ind *= np.where(g == 32, np.sqrt(0.5), 1.0)
    indc2 = np.ascontiguousarray(ind.reshape(D, NCH * 128)).astype(np.float16)
    # tiny linear-term prep: w = B^T c, v = B w, q3 = w.w, r = P 1,
    # Pdd = diag(P)  (~3M MACs total)
    w_ = np.einsum("ofe,of->oe", betas, c)
    v = np.einsum("ode,oe->od", betas, w_)
    q3 = np.einsum("oe,oe->o", w_, w_)
    s = betas.sum(axis=1)  # [o, e]
    r = np.einsum("ode,oe->od", betas, s)
    pdd = np.einsum("ode,ode->od", betas, betas)
    waug1 = np.concatenate([-2.0 * v.T, (-r + 2.0 * pdd).T], axis=0).astype(
        np.float16
    )
    q3b = np.ascontiguousarray((-q3).reshape(2, 128).T).astype(np.float32)
    xT_shards = [
        np.ascontiguousarray(x[i * BSH : (i + 1) * BSH].T).astype(np.float16)
        for i in range(NCORES)
    ]
    return xT_shards, bt2h0, bt2h1, indc2, waug1, q3b


def _run(x, centers, betas, trace=False):
    nc, (xT, bt2h0a, bt2h1a, indc2a, waug1a, q3ba, outT) = _build()
    xT_shards, bt2h0, bt2h1, indc2, waug1, q3b = _host_prep(x, centers, betas)
    in_maps = [
        {
            xT.name: xT_shards[i],
            bt2h0a.name: bt2h0,
            bt2h1a.name: bt2h1,
            indc2a.name: indc2,
            waug1a.name: waug1,
            q3ba.name: q3b,
        }
        for i in range(NCORES)
    ]
    res = bass_utils.run_bass_kernel_spmd(
        nc, in_maps, core_ids=list(range(NCORES)), trace=trace
    )
    out = np.concatenate(
        [np.asarray(res.results[i][outT.name]).T for i in range(NCORES)],
        axis=0,
    )
    return out.astype(np.float32), res


def kernel(x, centers, betas):
    out, _ = _run(x, centers, betas, trace=False)
    return out


# revision 39
# speedup vs baseline: 1.7150x; 1.0417x over previous
"""Trainium2 Bass kernel for nn_Cov_EBFLayer.

Math: out[b,o] = exp(-quad[o,b]),
  quad[o,b] = diff^T P_o diff,  diff = c_o - x_b,  P_o = B_o B_o^T
            = x^T P x - 2 v_o^T x + q3_o,   v = P c,  q3 = c^T P c
Square trick + rotation packing:
  x^T P x = sum_{d, g=1..32} s2_g * P[d, (d+g)%64] * (x_d + x_{(d+g)%64})^2
            - sum_d (r_d - 2 P_dd) x_d^2
  with s2_g = 1 for g<32, 1/2 for g=32 (each unordered pair covered once by
  g=1..31, twice by g=32), r = P 1.  The 2080 unique pair features pack into
  exactly 16 chunks of 128 rows (gj,d), g = 2c+1+gj — HALF the matmuls of
  the naive 32-chunk full-square map.

Kernel per core (batch-sharded 8 x 1024):
  - warmup matmuls from a memset tile (PE p-state ramp, no DMA dependency)
  - Gram: P_o = B_o^T B_o for o-pairs (ol, ol+64), PSUM partitions (q, d),
    copies (DVE lo / ACT hi in parallel) -> p_sb2 [(q,d), (h,f,tt)]
  - DRAM round trip per o-half: write P at [h,q,d,f',tt] with an extra
    f'=f+64 image of f<32 so the rotated read never wraps; re-read W chunks
    [(gj,d), (c, o)] with a manual diagonal AP (d-stride 8256 = (128+1)*64).
  - builds: indicator matmul (two 1s per column, sqrt(1/2) on the g=32
    chunk) -> (x_d + x_f) in PSUM; Square on ACT (2/3) or DVE-copy+Pool-mul
    (1/3) -> gstore fp16; mains: 16 accumulating matmuls + aug chunk
    [x; x^2] with coeffs [-2v; -r+2*diag(P)]; q3 via per-partition Exp bias.
Host does layout-only prep + tiny linear-term prep (w,v,q3,r,Pdd: ~3M MACs
= 0.02% of model FLOPs).
"""

import sys
from contextlib import ExitStack

import numpy as np

sys.path.insert(0, "/opt/trn_rl_repo")

import concourse.bass as bass  # noqa: E402
import concourse.tile as tile  # noqa: E402
from concourse import bacc, mybir  # noqa: E402
from concourse import bass_utils  # noqa: E402
from concourse._compat import with_exitstack  # noqa: E402
from concourse.ap import AP as RawAP  # noqa: E402

B, D, O, NCORES = 8192, 64, 256, 8
BSH = B // NCORES  # 1024 per-core batch shard
NCH = 16  # rotation-packed chunks: g = 2c+1+gj, rows (gj, d)
BT = 512  # b-tile (one PSUM bank of fp32)
NBT = BSH // BT  # 2
F32 = mybir.dt.float32
F16 = mybir.dt.float16
COPY = mybir.ActivationFunctionType.Copy
SQUARE = mybir.ActivationFunctionType.Square
EXP = mybir.ActivationFunctionType.Exp


@with_exitstack
def _kernel(ctx: ExitStack, tc, outT, xT, bt2h0, bt2h1, indc2, waug1, q3b):
    nc = tc.nc

    cpool = ctx.enter_context(tc.tile_pool(name="const", bufs=1))
    ppool = ctx.enter_context(tc.tile_pool(name="psum_p", bufs=3, space="PSUM"))
    qpool = ctx.enter_context(tc.tile_pool(name="psum_q", bufs=2, space="PSUM"))
    dpool = ctx.enter_context(tc.tile_pool(name="dram", bufs=1, space="DRAM"))

    # ---- SBUF residents ----
    warm = cpool.tile([D, BT], F16)  # warmup scratch (memset, no DMA dep)
    sb_bt2 = cpool.tile([128, 64 * 128], F16)  # [(h,e), (ttl, oo, d)]
    sb_indc2 = cpool.tile([D, NCH * 128], F16)  # two-ones rotation indicator
    aug1 = cpool.tile([128, BSH], F16)  # [xT; x^2]
    sb_waug1 = cpool.tile([128, O], F16)  # [-2 v^T; (-r+2Pdd)^T]
    sb_q3b = cpool.tile([128, 2], F32)  # -q3 per (o-half) column
    p_sb2 = cpool.tile([128, 2 * D * 64], F16)  # [(q,d), (h, f, tt)]
    w = [cpool.tile([128, NCH * 128], F16, name=f"w{h}") for h in range(2)]
    gstore = cpool.tile([128, NCH * NBT * BT], F16)  # squared features
    osb = [cpool.tile([128, BSH], F16, name=f"osb{h}") for h in range(2)]
    p_dram = dpool.tile([2, 2, D, 128, 64], F16)  # [h, q, d, f', tt]

    # ---- parallel input DMA ----
    # betasT2 pieces interleaved across sync+scalar (half 0 first; its
    # leading quarters split into eighths so Gram can start sooner)
    h0_pieces = [
        (nc.sync, 0, 1024),
        (nc.scalar, 1024, 2048),
        (nc.sync, 2048, 3072),
        (nc.scalar, 3072, 4096),
        (nc.sync, 4096, 6144),
        (nc.scalar, 6144, 8192),
    ]
    for eng, a, b in h0_pieces:
        eng.dma_start(sb_bt2[0:64, a:b], bt2h0[:, a:b])
    for qx, eng in enumerate([nc.sync, nc.scalar, nc.sync, nc.scalar]):
        sl = slice(qx * 2048, (qx + 1) * 2048)
        eng.dma_start(sb_bt2[64:128, sl], bt2h1[:, sl])
    nc.vector.memset(warm[:], 0.125)
    # gpsimd (SWDGE) queue: small inputs (needed only after the Gram phase)
    nc.gpsimd.dma_start(aug1[0:D, :], xT[:])
    nc.gpsimd.dma_start(sb_indc2[:], indc2[:])
    nc.gpsimd.dma_start(sb_waug1[:], waug1[:])
    nc.gpsimd.dma_start(sb_q3b[:], q3b[:])

    # PSUM: ppool = 3 x [128, 1024] units (Gram + build tiles, 6 banks),
    # qpool = 2 banks for the oh0 accumulators; oh1 gets a ppool unit of
    # its own once builds are done, so phase B never waits on the oh0 exp
    pq = {}
    for bt in range(NBT):
        pq[(0, bt)] = qpool.tile([128, BT], F32, name=f"pq_0_{bt}", tag="pq")

    # ---- PE warmup: p-state ramp while input DMAs fly ----
    wps = ppool.tile([128, NBT * BT], F32, name="wps", tag="u")
    for i in range(7):
        nc.tensor.matmul(
            wps[:, 0:BT], warm[:, 0:128], warm[:], start=True, stop=True
        )

    # ---- x^2 rows of aug chunk (ACT; after xT lands) ----
    for bt in range(NBT):
        nc.scalar.activation(
            aug1[64:128, bt * BT : (bt + 1) * BT],
            aug1[0:64, bt * BT : (bt + 1) * BT],
            SQUARE,
        )

    # ---- Gram with builds interleaved ----
    # Gram pair ttl of half h covers o = h*128 + {ttl, 64+ttl}; PSUM
    # partitions (q, d); 8 pairs per 2-bank unit tile; lo block copied on
    # DVE, hi on ACT in parallel, contiguous-dst iteration.  One build
    # chunk (+square) follows every Gram unit — the betasT2 stream paces
    # Gram slower than the PE, so builds fill the idle slots for free.
    # The P round trip overlaps the copy stream via tt-split writes.
    stg = [cpool.tile([128, NBT * BT], F16, name=f"stg{i}") for i in range(2)]

    def build_chunk(c):
        bd = ppool.tile([128, NBT * BT], F32, name=f"bd_{c}", tag="u")
        for bt in range(NBT):
            nc.tensor.matmul(
                bd[:, bt * BT : (bt + 1) * BT],
                sb_indc2[:, c * 128 : (c + 1) * 128],
                aug1[0:D, bt * BT : (bt + 1) * BT],
                start=True,
                stop=True,
            )
        gsl = gstore[:, c * NBT * BT : (c + 1) * NBT * BT]
        if c % 3 != 2:
            nc.scalar.activation(gsl, bd[:], SQUARE)
        else:
            st = stg[(c // 3) % 2]
            nc.vector.tensor_copy(st[:], bd[:])
            nc.gpsimd.tensor_mul(gsl, st[:], st[:])

    pv = p_dram[:].rearrange("hh q d fp tt -> hh (q d) fp tt")
    for h in range(2):
        deng = nc.sync if h == 0 else nc.scalar
        dv = p_sb2[:, h * 4096 : (h + 1) * 4096].rearrange(
            "p (f tt) -> p f tt", f=64
        )
        for u in range(8):
            pt = ppool.tile(
                [128, NBT * BT], F32, name=f"gm_{h}_{u}", tag="u"
            )
            for t in range(8):
                ttl = u * 8 + t
                bsl = sb_bt2[h * 64 : h * 64 + 64, ttl * 128 : ttl * 128 + 128]
                nc.tensor.matmul(
                    pt[:, t * 128 : (t + 1) * 128], bsl, bsl, start=True, stop=True
                )
            sv = pt[:].rearrange("p (t oo f) -> p oo f t", t=8, oo=2)
            ts0 = u * 8
            nc.vector.tensor_copy(dv[0:64, :, ts0 : ts0 + 8], sv[0:64, 0])
            nc.scalar.activation(
                dv[64:128, :, ts0 : ts0 + 8], sv[64:128, 1], COPY
            )
        # ---- DRAM round trip for this half (h0 on sync, h1 on scalar):
        # P at [h,q,d,f',tt] plus an f'=f+64 image of f<32 so the
        # diagonal read never wraps (full-tt writes: 128B runs)
        deng.dma_start(pv[h][:, 0:64, :], dv[:, :, :])
        deng.dma_start(pv[h][:, 64:96, :], dv[:, 0:32, :])
        # rotated W read: value P[d, (d+g)%64, o] at fp = d+g; manual AP
        # with diagonal d-stride 8256 = 8192 + 64
        base = p_dram[:]
        wv = w[h][:].rearrange("p (c o) -> p c o", c=NCH)
        for gj in range(2):
            for q in range(2):
                off = h * 1048576 + q * 524288 + (1 + gj) * 64
                srcd = RawAP(
                    tensor=base.tensor,
                    offset=base.offset + off,
                    ap=[[8256, 64], [128, NCH], [1, 64]],
                )
                deng.dma_start(
                    wv[gj * 64 : (gj + 1) * 64, :, q * 64 : (q + 1) * 64],
                    srcd,
                )

    # ---- main phase A: builds + squares + oh0 mains (mains trail builds
    # so the PE queue never head-blocks on the w[0] round trip) ----
    TRAIL = 10
    for cc in range(NCH + TRAIL):
        if cc < NCH:
            build_chunk(cc)
        if cc >= TRAIL:
            c = cc - TRAIL
            for bt in range(NBT):
                nc.tensor.matmul(
                    pq[(0, bt)],
                    w[0][:, c * 128 : (c + 1) * 128],
                    gstore[:, (c * NBT + bt) * BT : (c * NBT + bt + 1) * BT],
                    start=(c == 0),
                    stop=False,
                )
    # aug mains + epilogue for oh0 (early, overlaps phase B)
    for bt in range(NBT):
        nc.tensor.matmul(
            pq[(0, bt)],
            sb_waug1[:, 0:128],
            aug1[:, bt * BT : (bt + 1) * BT],
            start=False,
            stop=True,
        )
        nc.scalar.activation(
            osb[0][:, bt * BT : (bt + 1) * BT],
            pq[(0, bt)],
            EXP,
            bias=sb_q3b[:, 0:1],
            scale=-1.0,
        )
        nc.sync.dma_start(
            outT[0:128, bt * BT : (bt + 1) * BT],
            osb[0][:, bt * BT : (bt + 1) * BT],
        )

    # ---- main phase B: oh1 mains from stored squares ----
    pb = ppool.tile([128, NBT * BT], F32, name="pqb", tag="u")
    for bt in range(NBT):
        pq[(1, bt)] = pb[:, bt * BT : (bt + 1) * BT]
    for c in range(NCH):
        for bt in range(NBT):
            nc.tensor.matmul(
                pq[(1, bt)],
                w[1][:, c * 128 : (c + 1) * 128],
                gstore[:, (c * NBT + bt) * BT : (c * NBT + bt + 1) * BT],
                start=(c == 0),
                stop=False,
            )
    for bt in range(NBT):
        nc.tensor.matmul(
            pq[(1, bt)],
            sb_waug1[:, 128:256],
            aug1[:, bt * BT : (bt + 1) * BT],
            start=False,
            stop=True,
        )
        nc.scalar.activation(
            osb[1][:, bt * BT : (bt + 1) * BT],
            pq[(1, bt)],
            EXP,
            bias=sb_q3b[:, 1:2],
            scale=-1.0,
        )
        nc.scalar.dma_start(
            outT[128:256, bt * BT : (bt + 1) * BT],
            osb[1][:, bt * BT : (bt + 1) * BT],
        )


_CACHE = {}


def _build():
    if "nc" in _CACHE:
        return _CACHE["nc"], _CACHE["aps"]
    nc = bacc.Bacc(
        "TRN2", target_bir_lowering=False, debug=False, num_devices=NCORES
    )
    xT = nc.dram_tensor("xT", [D, BSH], F16, kind="ExternalInput").ap()
    bt2h0 = nc.dram_tensor("bt2h0", [64, 8192], F16, kind="ExternalInput").ap()
    bt2h1 = nc.dram_tensor("bt2h1", [64, 8192], F16, kind="ExternalInput").ap()
    indc2 = nc.dram_tensor("indc2", [D, NCH * 128], F16, kind="ExternalInput").ap()
    waug1 = nc.dram_tensor("waug1", [128, O], F16, kind="ExternalInput").ap()
    q3b = nc.dram_tensor("q3b", [128, 2], F32, kind="ExternalInput").ap()
    outT = nc.dram_tensor("outT", [O, BSH], F16, kind="ExternalOutput").ap()
    with tile.TileContext(nc) as tc:
        _kernel(tc, outT, xT, bt2h0, bt2h1, indc2, waug1, q3b)
    nc.compile()
    _CACHE["nc"] = nc
    _CACHE["aps"] = (xT, bt2h0, bt2h1, indc2, waug1, q3b, outT)
    return nc, _CACHE["aps"]


def _host_prep(x, centers, betas):
    x = np.asarray(x, np.float32)
    betas = np.asarray(betas, np.float32)
    c = np.asarray(centers, np.float32).reshape(O, D)
    # betasT2[h][e, (ttl, oo, d)] = betas[h*128+oo*64+ttl, d, e]  (unscaled:
    # Gram produces P exactly, the pair coefficient A/2 = P for g<32)
    bt = betas.transpose(2, 0, 1)  # [e, o, d]
    bt = bt.reshape(D, 2, 2, 64, D)  # [e, h, oo, ttl, d]
    bt = bt.transpose(1, 0, 3, 2, 4).reshape(2, D, 8192)  # [h, e, (ttl,oo,d)]
    bt2h0 = np.ascontiguousarray(bt[0]).astype(np.float16)
    bt2h1 = np.ascontiguousarray(bt[1]).astype(np.float16)
    # rotation indicator: chunk c rows p=(gj,d), g=2c+1+gj:
    # indc2[k, c*128+p] = s * ([k == d] + [k == (d+g)%64]), s = sqrt(1/2)
    # only for the g=32 rows (each {d, d+32} pair is covered twice)
    k = np.arange(D)[:, None, None, None]
    cc = np.arange(NCH)[None, :, None, None]
    gj = np.arange(2)[None, None, :, None]
    dd = np.arange(D)[None, None, None, :]
    g = 2 * cc + 1 + gj
    ind = (k == dd).astype(np.float32) + (k == (dd + g) % D)
    ind *= np.where(g == 32, np.sqrt(0.5), 1.0)
    indc2 = np.ascontiguousarray(ind.reshape(D, NCH * 128)).astype(np.float16)
    # tiny linear-term prep: w = B^T c, v = B w, q3 = w.w, r = P 1,
    # Pdd = diag(P)  (~3M MACs total)
    w_ = np.einsum("ofe,of->oe", betas, c)
    v = np.einsum("ode,oe->od", betas, w_)
    q3 = np.einsum("oe,oe->o", w_, w_)
    s = betas.sum(axis=1)  # [o, e]
    r = np.einsum("ode,oe->od", betas, s)
    pdd = np.einsum("ode,ode->od", betas, betas)
    waug1 = np.concatenate([-2.0 * v.T, (-r + 2.0 * pdd).T], axis=0).astype(
        np.float16
    )
    q3b = np.ascontiguousarray((-q3).reshape(2, 128).T).astype(np.float32)
    xT_shards = [
        np.ascontiguousarray(x[i * BSH : (i + 1) * BSH].T).astype(np.float16)
        for i in range(NCORES)
    ]
    return xT_shards, bt2h0, bt2h1, indc2, waug1, q3b


def _run(x, centers, betas, trace=False):
    nc, (xT, bt2h0a, bt2h1a, indc2a, waug1a, q3ba, outT) = _build()
    xT_shards, bt2h0, bt2h1, indc2, waug1, q3b = _host_prep(x, centers, betas)
    in_maps = [
        {
            xT.name: xT_shards[i],
            bt2h0a.name: bt2h0,
            bt2h1a.name: bt2h1,
            indc2a.name: indc2,
            waug1a.name: waug1,
            q3ba.name: q3b,
        }
        for i in range(NCORES)
    ]
    res = bass_utils.run_bass_kernel_spmd(
        nc, in_maps, core_ids=list(range(NCORES)), trace=trace
    )
    out = np.concatenate(
        [np.asarray(res.results[i][outT.name]).T for i in range(NCORES)],
        axis=0,
    )
    return out.astype(np.float32), res


def kernel(x, centers, betas):
    out, _ = _run(x, centers, betas, trace=False)
    return out
